# revision 34
# baseline (speedup 1.0000x reference)
"""Contrastive loss (CLIP-style BCE) on 8 Trainium2 NeuronCores.

Strategy: data-parallel over the batch dim. Each core takes a 128-row shard
of img_features (75.5 MB) with batch rows mapped to the 128 SBUF
partitions, so every pooling DMA moves a [128, ncb*576] tile whose
per-partition source is one contiguous run — large descriptors, single
HWDGE ring, ~full streaming rate. The [128, 1024] logits slice, BCE
partials, and the 128-way reduction all run per core; the host sums the 8
partial scalars and divides by B*B.

The kernel is a single DMA stream (75.5 MB img + 1 MB text + 4 KB labels
per core) with everything else hidden behind it:
- The FIRST thing issued is img tile 0 — text/labels follow inside the
  stream, so the DMA engines never idle at the head.
- Labels arrive as [1,128] / [1,1024] rows (one descriptor each) and are
  broadcast on-chip with K=1 PE matmuls; no [128,B] broadcast DMA.
- Text arrives as ONE strided DMA ([128, 8, 256] view of [1024, 256]).
- The gram contraction is split into three channel chunks (128/112/16)
  whose transposes + matmul accumulations run AS the channels finish
  pooling, so after the last (1-channel) img tile only a [16,...]
  transpose + 2 psum-accumulate matmuls + softplus tail remain.
- Matmul operands are cast to bf16 (4x PE throughput vs fp32; psum
  accumulation stays fp32, and the loss tolerance is 2e-2).
- The pooled-row norm accumulates per-tile (Square accum_out into
  ns_parts), so the tail only combines 35 partials + one rsqrt chain.

One activation-table set for the whole kernel: bacc's table-load pass
maps each function to the FIRST act_info.json set containing it, which
ping-pongs natural_log <-> exp_and_others around every rsqrt (1.28 us
per reload). We patch bacc's view of the tables to subtract
natural_log_exp_and_others' functions from sets listed before it; every
function then first-matches that one set. Set ids and real table
contents are unchanged.

Runtime notes (bisected on this axon/fakenrt stack by a prior session):
- PE is_transpose matmuls and InstTensorTensorReduce crash the exec unit;
  transposes are done as regular matmuls against identity.
- float32r matmul operands and SWDGE accumulate/cast DMAs all measured
  SLOWER on HW than the plain-f32 HWDGE pipeline, despite the cost model
  predicting otherwise.
- Softplus/Sqrt have no single-table path here; softplus = ln(exp(x)+1)
  (|x| <= 1/0.07 so exp is safe), rsqrt = exp(-0.5*ln(x)) + one Newton
  step.
- max_dma_last_dim=2304 splits each 8-channel partition line into two
  9216-B descriptors; the finer SDMA interleave measured -9.4 us/rep.
"""

import numpy as np

import concourse.bacc as bacc
import concourse.hw_specs as hw_specs
import concourse.mybir as mybir
import concourse.tile as tile
from concourse.bass_utils import run_bass_kernel_spmd
from concourse.masks import make_identity

_PREF_SET = "natural_log_exp_and_others"


def _pinned_tables(arch):
    tabs = hw_specs.get_activation_tables(arch)
    try:
        pref = tabs.get(_PREF_SET)
        if pref is None:
            return tabs
        out = {}
        seen_pref = False
        for name, funcs in tabs.items():
            if name == _PREF_SET:
                out[name] = funcs
                seen_pref = True
            else:
                out[name] = funcs if seen_pref else (funcs - pref)
        return out
    except Exception:
        return tabs


bacc.get_activation_tables = _pinned_tables

N_CORES = 8
B, C, H, W = 1024, 256, 24, 24
HW = H * W  # 576
BS = B // N_CORES  # 128 rows per core
P = 128
TEMP = 0.07
INV_TEMP = 1.0 / TEMP
NT = B // P  # 8 text tiles
ACT_SET = frozenset({1, 3, 5, 7, 9, 11, 13, 15})  # c % 16 -> ACT reduce (8/16)

F32 = mybir.dt.float32
BF16 = mybir.dt.bfloat16
ALU = mybir.AluOpType
ACT = mybir.ActivationFunctionType
AX = mybir.AxisListType

_NC_CACHE = []


def _emit_newton(nc, small, y0, ns, out_rv, tag, width):
    """out_rv = y0 * (1.5 - 0.5 * ns * y0^2) — Newton step for rsqrt."""
    t1 = small.tile([P, width], F32, tag=f"{tag}_t1", name=f"{tag}_t1")
    nc.vector.tensor_mul(t1, y0, y0)
    nc.vector.tensor_mul(t1, t1, ns)
    nc.vector.tensor_scalar(
        out=t1, in0=t1, scalar1=-0.5, scalar2=1.5, op0=ALU.mult, op1=ALU.add
    )
    nc.vector.tensor_mul(out_rv, y0, t1)


def _emit_body(nc, pools, identity_bf, ones_row, one1, img, txt, lab_row, lab_all, out, cfg):
    consts, big, ascrp, txtp, small, persist, psum_tp, psum_g, psum_lab, psum_warm = pools
    act_set = cfg.get("act_set", ACT_SET)
    sizes = cfg.get("sizes")
    if sizes is None:
        sizes = [8] * 30 + [8, 4, 2, 1, 1]
    assert sum(sizes) == C
    ntiles = len(sizes)
    # gram contraction chunk boundaries (channel index); must land on tile
    # boundaries, and each boundary mod 128 must be 0/32/64 (PE base
    # partition rule). After each boundary's channels are pooled, that
    # chunk's transpose + psum-accumulate matmuls are emitted mid-stream.
    chunks = cfg.get("chunks", [0, 128, 192, 256])
    assert chunks[0] == 0 and chunks[-1] == C
    # (gpsimd tensor_reduce only supports cross-partition axes, so the Pool
    # engine cannot take free-axis channel reduces; keep this empty)
    pool_channels = frozenset(cfg.get("pool_channels", ()))

    pooled = persist.tile([P, C], F32, tag="pooled", name="pooled")
    pooled_bf = persist.tile([P, C], BF16, tag="pooled_bf", name="pooled_bf")
    ns_parts = small.tile([P, ntiles], F32, tag="ns_parts", name="ns_parts")
    tgt01 = persist.tile([P, B], BF16, tag="tgt01", name="tgt01")
    tgtT = persist.tile([P, NT, P], BF16, tag="tgtT", name="tgtT")
    w_sb = persist.tile([P, C], F32, tag="w_sb", name="w_sb")
    txt_sb = txtp.tile([P, NT, C], F32, tag="ttin", name="ttin")
    txt_bf = txtp.tile([P, NT, C], BF16, tag="ttbf", name="ttbf")
    txtT = [
        persist.tile([P, B], BF16, tag=f"txtT{cb}", name=f"txtT{cb}") for cb in range(2)
    ]
    pnT = [
        persist.tile([P, P], BF16, tag=f"pnT{k}", name=f"pnT{k}")
        for k in range(len(chunks) - 1)
    ]
    lab_bf = small.tile([1, B], BF16, tag="lab_bf", name="lab_bf")
    labr_bf = small.tile([1, P], BF16, tag="labr_bf", name="labr_bf")
    lab_row_sb = small.tile([P, 1], F32, tag="lab_row_sb", name="lab_row_sb")
    tns = small.tile([P, NT], F32, tag="tns", name="tns")
    ty0 = small.tile([P, NT], F32, tag="ty0", name="ty0")
    trv = small.tile([P, NT], F32, tag="trv", name="trv")
    xt_scr = small.tile([P, C], F32, tag="xt_scr", name="xt_scr")
    gs = []

    chunk_idx = 1  # next chunk boundary to emit gram work for

    def emit_gram_chunk(k):
        """Transpose pooled channels [chunks[k-1], chunks[k]) and accumulate
        their gram contribution into the psum g tiles."""
        c0, c1 = chunks[k - 1], chunks[k]
        w = c1 - c0
        # both matmul operands must share a base partition, so the chunk's
        # transpose lands at its natural offset within the 128-channel block
        cb = c0 // P
        r0 = c0 - cb * P
        pt = psum_tp.tile([P, P], F32, tag="pt", name="pt")
        nc.tensor.matmul(
            pt[r0 : r0 + w, :], pooled_bf[:, c0:c1], identity_bf, start=True, stop=True
        )
        last = k == len(chunks) - 1
        nc.vector.tensor_copy(pnT[k - 1][r0 : r0 + w, :], pt[r0 : r0 + w, :])
        if last and cfg.get("gap_filler", False):
            pw = psum_lab.tile([P, 512], F32, tag="pl", name="pl")
            nc.tensor.matmul(
                pw[:, :256], txtT[0][:, :P], txtT[0][:, :256], start=True, stop=True
            )
        first = k == 1
        for nbk in range(2):
            if first:
                g = psum_g.tile([P, 512], F32, tag=f"g{nbk}", name=f"g{nbk}")
                gs.append(g)
            nc.tensor.matmul(
                gs[nbk],
                pnT[k - 1][r0 : r0 + w, :],
                txtT[cb][r0 : r0 + w, nbk * 512 : (nbk + 1) * 512],
                start=first,
                stop=last,
            )

    # ---- per-tile extra work, keyed by tile index (emitted after that
    # tile's reduces so each engine queue stays ahead of the stream) ----
    def extra_work(t):
        if t == 0:
            # labels: on-chip broadcast. lab row already cast to bf16 below.
            nc.vector.tensor_copy(lab_bf, lab_all_sb)
            nc.vector.tensor_copy(labr_bf, lab_row_sb_row)
            pr = psum_tp.tile([P, P], F32, tag="pt", name="pt")
            nc.tensor.matmul(pr[:, :1], labr_bf, one1, start=True, stop=True)
            nc.vector.tensor_copy(lab_row_sb, pr[:, :1])
            for h in range(2):
                pl = psum_lab.tile([P, 512], F32, tag="pl", name="pl")
                nc.tensor.matmul(
                    pl, ones_row, lab_bf[:, h * 512 : (h + 1) * 512],
                    start=True, stop=True,
                )
                # tgt01 = (lab == lab_row); {0,1} so exact in bf16
                nc.vector.tensor_scalar(
                    out=tgt01[:, h * 512 : (h + 1) * 512],
                    in0=pl,
                    scalar1=lab_row_sb,
                    scalar2=1.0,
                    op0=ALU.is_equal,
                    op1=ALU.mult,
                )
        elif 1 <= t <= 4:
            # text norms, 2 tiles per img tile
            for tb in (2 * t - 2, 2 * t - 1):
                tsq = txtp.tile([P, C], F32, tag="tsq", name="tsq")
                nc.vector.tensor_mul(tsq, txt_sb[:, tb, :], txt_sb[:, tb, :])
                nc.vector.reduce_sum(out=tns[:, tb : tb + 1], in_=tsq, axis=AX.X)
        elif t == 5:
            # batched rsqrt for all 8 text tiles
            nc.scalar.activation(ty0, tns, ACT.Ln)
            nc.scalar.activation(ty0, ty0, ACT.Exp, scale=-0.5)
            _emit_newton(nc, small, ty0, tns, trv, "trsq", NT)
        elif 6 <= t <= 9:
            for tb in (2 * t - 12, 2 * t - 11):
                nc.vector.tensor_scalar_mul(
                    txt_bf[:, tb, :], txt_sb[:, tb, :], trv[:, tb : tb + 1]
                )
        elif 10 <= t <= 12:
            # text transposes: 16 matmuls over 3 tiles
            lo = (16 * (t - 10)) // 3
            hi = (16 * (t - 9)) // 3
            for i in range(lo, hi):
                tb, cb = divmod(i, 2)
                pt = psum_tp.tile([P, P], F32, tag="pt", name="pt")
                nc.tensor.matmul(
                    pt,
                    txt_bf[:, tb, cb * P : (cb + 1) * P],
                    identity_bf,
                    start=True,
                    stop=True,
                )
                nc.vector.tensor_copy(txtT[cb][:, tb * P : (tb + 1) * P], pt)
        elif t == 13:
            # transpose the target mask: tgtT[:, jt, b] = tgt01[b, jt*128+...]
            for jt in range(NT):
                pt = psum_tp.tile([P, P], F32, tag="pt", name="pt")
                nc.tensor.matmul(
                    pt,
                    tgt01[:, jt * P : (jt + 1) * P],
                    identity_bf,
                    start=True,
                    stop=True,
                )
                nc.vector.tensor_copy(tgtT[:, jt, :], pt)
        elif t == 14:
            # W[b,c] = sum_j tgt01[b,j] * txt_n[j,c]: needs NO img data, so it
            # runs mid-stream and takes the whole x*z term off the tail (the
            # tail then only needs sum_c pooled[b,c]*W[b,c]).
            pw = psum_warm.tile([P, C], F32, tag="W", name="W")
            for jt in range(NT):
                nc.tensor.matmul(
                    pw, tgtT[:, jt, :], txt_bf[:, jt, :],
                    start=(jt == 0), stop=(jt == NT - 1),
                )
            nc.vector.tensor_copy(w_sb, pw)

    # ---- the stream ----
    c = 0
    for t, sz in enumerate(sizes):
        it = big.tile([P, sz, HW], F32, tag="imgin", name="imgin")
        if t == ntiles - 1 and sz == 1 and cfg.get("split_last", True):
            # final channel arrives as two half-rows so the tail reduce can
            # start one half-transfer earlier
            nc.sync.dma_start(out=it[:, :, : HW // 2], in_=img[:, c : c + 1, : HW // 2])
            nc.sync.dma_start(out=it[:, :, HW // 2 :], in_=img[:, c : c + 1, HW // 2 :])
        else:
            nc.sync.dma_start(out=it, in_=img[:, c : c + sz, :], max_dma_last_dim=2304)
        if t == 0:
            # small inputs ride the stream behind tile 0
            # text + labels issue from the ACT ring: keeps the sync ring's
            # serial descriptor-issue pipeline exclusively for the img stream
            # (-4.6 us on the v1/CoreSim clock, neutral on TimelineSim)
            use_side = cfg.get("side_ring", True)
            side_t = nc.scalar if use_side in ("both", "txt", True) else nc.sync
            side_l = nc.scalar if use_side in ("both", "lab", True) else nc.sync
            if use_side == "lab3":
                side_t, side_l = nc.scalar, nc.vector
            side_t.dma_start(
                out=txt_sb, in_=txt.rearrange("(t p) c -> p t c", p=P)
            )
            lab_all_sb = small.tile([1, B], F32, tag="lab_all_sb", name="lab_all_sb")
            side_l.dma_start(out=lab_all_sb, in_=lab_all)
            lab_row_sb_row = small.tile([1, P], F32, tag="lab_row_r", name="lab_row_r")
            side_l.dma_start(out=lab_row_sb_row, in_=lab_row)
        c0 = c
        for j in range(sz):
            chunk = it[:, j, :]
            # the very last channel reduce goes to DVE: its queue is empty at
            # the stream end, while ACT still owes squares + the rsqrt chain
            if c in pool_channels:
                nc.gpsimd.reduce_sum(out=pooled[:, c : c + 1], in_=chunk, axis=AX.X)
            elif (c % 16) in act_set and c != C - 1:
                ascr = ascrp.tile([P, HW], F32, tag="ascr", name="ascr")
                nc.scalar.activation(
                    ascr, chunk, ACT.Identity, accum_out=pooled[:, c : c + 1]
                )
            elif c == C - 1 and t == ntiles - 1 and sz == 1:
                ph = small.tile([P, 2], F32, tag="ph", name="ph")
                nc.vector.reduce_sum(out=ph[:, :1], in_=chunk[:, : HW // 2], axis=AX.X)
                nc.vector.reduce_sum(out=ph[:, 1:], in_=chunk[:, HW // 2 :], axis=AX.X)
                nc.vector.reduce_sum(out=pooled[:, c : c + 1], in_=ph, axis=AX.X)
            else:
                nc.vector.reduce_sum(out=pooled[:, c : c + 1], in_=chunk, axis=AX.X)
            c += 1
        # cast this tile's channels for the gram matmuls + norm partial
        nc.vector.tensor_copy(pooled_bf[:, c0:c], pooled[:, c0:c])
        sq_scr = ascrp.tile([P, sz], F32, tag="sq_scr", name="sq_scr")
        nc.scalar.activation(
            sq_scr, pooled[:, c0:c], ACT.Square, accum_out=ns_parts[:, t : t + 1]
        )
        extra_work(t)
        if t == cfg.get("warm_tile", ntiles - 4) and cfg.get("warm", False):
            # PE p-state warmers: two throwaway fp32 matmuls anchored on THIS
            # tile's freshly-DMA'd data, so they execute ~4us before the tail
            # matmuls and hold the PE ramp past the 3us full-speed threshold
            # (cold tail matmuls cost 427+788 ns; warm ones 213 each).
            for wi in range(2):
                pw = psum_lab.tile([P, 512], F32, tag="pl", name="pl")
                nc.tensor.matmul(
                    pw, it[:, 0, :P], it[:, 0, :512], start=True, stop=True
                )
        while chunk_idx < len(chunks) and c >= chunks[chunk_idx]:
            emit_gram_chunk(chunk_idx)
            chunk_idx += 1

    # ---- tail: pooled-row rsqrt + softplus/target accumulation ----
    ns = small.tile([P, 1], F32, tag="ns", name="ns")
    nc.vector.reduce_sum(out=ns, in_=ns_parts, axis=AX.X)
    # rsqrt via exp(-0.5*ln(ns)); no Newton step here — it sits on the tail's
    # critical path and the executor's Ln/Exp leave ~1e-5 rel error, far
    # inside the 2e-2 gate (the off-critical text rsqrt keeps its Newton).
    rv = small.tile([P, 1], F32, tag="rv", name="rv")
    nc.scalar.activation(rv, ns, ACT.Ln)
    nc.scalar.activation(rv, rv, ACT.Exp, scale=-0.5)
    rv_sc = small.tile([P, 1], F32, tag="rv_sc", name="rv_sc")
    nc.vector.tensor_scalar_mul(rv_sc, rv, INV_TEMP)

    # fin[:,0] = sum_j softplus(logits); fin[:,1] = sum_c pooled*W (raw x*z
    # term); fin[:,2] = rv. fin goes to HBM as-is; the host computes
    # sum(fin0) - (1/T)*sum(fin1*fin2) — exact, and it decouples the output
    # from the rsqrt chain while dropping an op from the tail.
    fin = small.tile([P, 3], F32, tag="fin", name="fin")
    nc.vector.tensor_copy(fin[:, 2:], rv)
    if cfg.get("escr_psum", False):
        e_scr = psum_warm.tile([P, B], F32, tag="e_scr", name="e_scr")
    else:
        e_scr = small.tile([P, B], F32, tag="e_scr", name="e_scr")
    for nbk in range(2):
        # softplus(x) = ln(exp(x) + 1); |x| <= 1/0.07 so exp can't overflow
        nc.scalar.activation(
            e_scr[:, nbk * 512 : (nbk + 1) * 512], gs[nbk], ACT.Exp, scale=rv_sc
        )
    # one wide Ln over both halves: saves an accumulator drain + op overhead
    if cfg.get("probe_noln", False):
        nc.vector.reduce_sum(out=fin[:, :1], in_=e_scr, axis=AX.X)
    else:
        nc.scalar.activation(e_scr, e_scr, ACT.Ln, bias=1.0, accum_out=fin[:, :1])
    if cfg.get("split_out", False):
        nc.sync.dma_start(out=out[:, 1:], in_=fin[:, 1:])
        nc.sync.dma_start(out=out[:, :1], in_=fin[:, :1])

    # emitted AFTER the Ln so the DVE ready-first scheduler prefers the
    # critical pnT copy over this off-path work at the stream tail
    nc.vector.tensor_mul(xt_scr, pooled, w_sb)
    nc.vector.reduce_sum(out=fin[:, 1:2], in_=xt_scr, axis=AX.X)
    if not cfg.get("probe_noout", False) and not cfg.get("split_out", False):
        out_q = nc.scalar if cfg.get("out_ring_act", False) else nc.sync
        out_q.dma_start(out=out, in_=fin)


def _build_nc(reps=1, **cfg):
    nc = bacc.Bacc("TRN2", target_bir_lowering=False, debug=False, num_devices=N_CORES)
    img = nc.dram_tensor("img", [BS, C, HW], F32, kind="ExternalInput").ap()
    txt = nc.dram_tensor("txt", [B, C], F32, kind="ExternalInput").ap()
    lab_row = nc.dram_tensor("lab_row", [1, BS], F32, kind="ExternalInput").ap()
    lab_all = nc.dram_tensor("lab_all", [1, B], F32, kind="ExternalInput").ap()
    outs = [
        nc.dram_tensor(
            "partial" if r == 0 else f"partial{r}", [P, 3], F32, kind="ExternalOutput"
        ).ap()
        for r in range(reps)
    ]

    with tile.TileContext(nc) as tc:
        with (
            tc.tile_pool(name="consts", bufs=1) as consts,
            tc.tile_pool(name="big", bufs=cfg.get("big_bufs", 6)) as big,
            tc.tile_pool(name="ascrp", bufs=2) as ascrp,
            tc.tile_pool(name="txtp", bufs=1) as txtp,
            tc.tile_pool(name="small", bufs=2) as small,
            tc.tile_pool(name="persist", bufs=1) as persist,
            tc.tile_pool(name="psum_tp", bufs=2, space="PSUM") as psum_tp,
            tc.tile_pool(name="psum_g", bufs=1, space="PSUM") as psum_g,
            tc.tile_pool(name="psum_lab", bufs=1, space="PSUM") as psum_lab,
            tc.tile_pool(name="psum_warm", bufs=1, space="PSUM") as psum_warm,
        ):
            identity_bf = consts.tile([P, P], BF16, tag="identity")
            make_identity(nc, identity_bf)
            ones_row = consts.tile([1, P], BF16, tag="ones_row")
            nc.vector.memset(ones_row, 1.0)
            one1 = consts.tile([1, 1], BF16, tag="one1")
            nc.vector.memset(one1, 1.0)
            pools = (consts, big, ascrp, txtp, small, persist, psum_tp, psum_g, psum_lab, psum_warm)
            for r in range(reps):
                _emit_body(
                    nc, pools, identity_bf, ones_row, one1,
                    img, txt, lab_row, lab_all, outs[r], cfg,
                )

    nc.finalize()
    return nc


def _get_nc():
    if not _NC_CACHE:
        _NC_CACHE.append(_build_nc())
    return _NC_CACHE[0]


def make_in_maps(img_features, text_embeds, labels_f):
    img3 = img_features.reshape(B, C, HW)
    in_maps = []
    for i in range(N_CORES):
        sl = slice(i * BS, (i + 1) * BS)
        in_maps.append(
            {
                "img": img3[sl],
                "txt": text_embeds,
                "lab_row": labels_f[sl].reshape(1, BS),
                "lab_all": labels_f.reshape(1, B),
            }
        )
    return in_maps


def kernel(img_features, text_embeds, labels):
    img_features = np.ascontiguousarray(np.asarray(img_features, dtype=np.float32))
    text_embeds = np.ascontiguousarray(np.asarray(text_embeds, dtype=np.float32))
    labels_f = np.asarray(labels).astype(np.float32)  # values < 16: exact in f32

    nc = _get_nc()
    in_maps = make_in_maps(img_features, text_embeds, labels_f)
    r = run_bass_kernel_spmd(nc, in_maps, core_ids=list(range(N_CORES)))
    total = 0.0
    for i in range(N_CORES):
        p = r.results[i]["partial"].astype(np.float64)
        total += p[:, 0].sum() - INV_TEMP * float(p[:, 1] @ p[:, 2])
    return np.float32(total / (B * B))


# revision 38
# speedup vs baseline: 1.0006x; 1.0006x over previous
"""Contrastive loss (CLIP-style BCE) on 8 Trainium2 NeuronCores.

Strategy: data-parallel over the batch dim. Each core takes a 128-row shard
of img_features (75.5 MB) with batch rows mapped to the 128 SBUF
partitions, so every pooling DMA moves a [128, ncb*576] tile whose
per-partition source is one contiguous run — large descriptors, single
HWDGE ring, ~full streaming rate. The [128, 1024] logits slice, BCE
partials, and the 128-way reduction all run per core; the host sums the 8
partial scalars and divides by B*B.

The kernel is a single DMA stream (75.5 MB img + 1 MB text + 4 KB labels
per core) with everything else hidden behind it:
- The FIRST thing issued is img tile 0 — text/labels follow inside the
  stream, so the DMA engines never idle at the head.
- Labels arrive as [1,128] / [1,1024] rows (one descriptor each) and are
  broadcast on-chip with K=1 PE matmuls; no [128,B] broadcast DMA.
- Text arrives as ONE strided DMA ([128, 8, 256] view of [1024, 256]).
- The gram contraction is split into three channel chunks (128/112/16)
  whose transposes + matmul accumulations run AS the channels finish
  pooling, so after the last (1-channel) img tile only a [16,...]
  transpose + 2 psum-accumulate matmuls + softplus tail remain.
- Matmul operands are cast to bf16 (4x PE throughput vs fp32; psum
  accumulation stays fp32, and the loss tolerance is 2e-2).
- The pooled-row norm accumulates per-tile (Square accum_out into
  ns_parts), so the tail only combines 35 partials + one rsqrt chain.

One activation-table set for the whole kernel: bacc's table-load pass
maps each function to the FIRST act_info.json set containing it, which
ping-pongs natural_log <-> exp_and_others around every rsqrt (1.28 us
per reload). We patch bacc's view of the tables to subtract
natural_log_exp_and_others' functions from sets listed before it; every
function then first-matches that one set. Set ids and real table
contents are unchanged.

Runtime notes (bisected on this axon/fakenrt stack by a prior session):
- PE is_transpose matmuls and InstTensorTensorReduce crash the exec unit;
  transposes are done as regular matmuls against identity.
- float32r matmul operands and SWDGE accumulate/cast DMAs all measured
  SLOWER on HW than the plain-f32 HWDGE pipeline, despite the cost model
  predicting otherwise.
- Softplus/Sqrt have no single-table path here; softplus = ln(exp(x)+1)
  (|x| <= 1/0.07 so exp is safe), rsqrt = exp(-0.5*ln(x)) + one Newton
  step.
- max_dma_last_dim=2304 splits each 8-channel partition line into two
  9216-B descriptors; the finer SDMA interleave measured -9.4 us/rep.
"""

import numpy as np

import concourse.bacc as bacc
import concourse.hw_specs as hw_specs
import concourse.mybir as mybir
import concourse.tile as tile
from concourse.bass_utils import run_bass_kernel_spmd
from concourse.masks import make_identity

_PREF_SET = "natural_log_exp_and_others"


def _pinned_tables(arch):
    tabs = hw_specs.get_activation_tables(arch)
    try:
        pref = tabs.get(_PREF_SET)
        if pref is None:
            return tabs
        out = {}
        seen_pref = False
        for name, funcs in tabs.items():
            if name == _PREF_SET:
                out[name] = funcs
                seen_pref = True
            else:
                out[name] = funcs if seen_pref else (funcs - pref)
        return out
    except Exception:
        return tabs


bacc.get_activation_tables = _pinned_tables

N_CORES = 8
B, C, H, W = 1024, 256, 24, 24
HW = H * W  # 576
BS = B // N_CORES  # 128 rows per core
P = 128
TEMP = 0.07
INV_TEMP = 1.0 / TEMP
NT = B // P  # 8 text tiles
ACT_SET = frozenset({1, 3, 5, 7, 9, 11, 13, 15})  # c % 16 -> ACT reduce (8/16)

F32 = mybir.dt.float32
BF16 = mybir.dt.bfloat16
ALU = mybir.AluOpType
ACT = mybir.ActivationFunctionType
AX = mybir.AxisListType

_NC_CACHE = []


def _emit_newton(nc, small, y0, ns, out_rv, tag, width):
    """out_rv = y0 * (1.5 - 0.5 * ns * y0^2) — Newton step for rsqrt."""
    t1 = small.tile([P, width], F32, tag=f"{tag}_t1", name=f"{tag}_t1")
    nc.vector.tensor_mul(t1, y0, y0)
    nc.vector.tensor_mul(t1, t1, ns)
    nc.vector.tensor_scalar(
        out=t1, in0=t1, scalar1=-0.5, scalar2=1.5, op0=ALU.mult, op1=ALU.add
    )
    nc.vector.tensor_mul(out_rv, y0, t1)


def _emit_body(nc, pools, identity_bf, ones_row, one1, img, txt, lab_row, lab_all, out, cfg):
    consts, big, ascrp, txtp, small, persist, psum_tp, psum_g, psum_lab, psum_warm = pools
    act_set = cfg.get("act_set", ACT_SET)
    sizes = cfg.get("sizes")
    if sizes is None:
        sizes = [8] * 30 + [8, 4, 2, 1, 1]
    assert sum(sizes) == C
    ntiles = len(sizes)
    # gram contraction chunk boundaries (channel index); must land on tile
    # boundaries, and each boundary mod 128 must be 0/32/64 (PE base
    # partition rule). After each boundary's channels are pooled, that
    # chunk's transpose + psum-accumulate matmuls are emitted mid-stream.
    chunks = cfg.get("chunks", [0, 128, 192, 256])
    assert chunks[0] == 0 and chunks[-1] == C
    # (gpsimd tensor_reduce only supports cross-partition axes, so the Pool
    # engine cannot take free-axis channel reduces; keep this empty)
    pool_channels = frozenset(cfg.get("pool_channels", ()))

    pooled = persist.tile([P, C], F32, tag="pooled", name="pooled")
    pooled_bf = persist.tile([P, C], BF16, tag="pooled_bf", name="pooled_bf")
    ns_parts = small.tile([P, ntiles], F32, tag="ns_parts", name="ns_parts")
    tgt01 = persist.tile([P, B], BF16, tag="tgt01", name="tgt01")
    tgtT = persist.tile([P, NT, P], BF16, tag="tgtT", name="tgtT")
    w_sb = persist.tile([P, C], F32, tag="w_sb", name="w_sb")
    txt_sb = txtp.tile([P, NT, C], F32, tag="ttin", name="ttin")
    txt_bf = txtp.tile([P, NT, C], BF16, tag="ttbf", name="ttbf")
    txtT = [
        persist.tile([P, B], BF16, tag=f"txtT{cb}", name=f"txtT{cb}") for cb in range(2)
    ]
    pnT = [
        persist.tile([P, P], BF16, tag=f"pnT{k}", name=f"pnT{k}")
        for k in range(len(chunks) - 1)
    ]
    lab_bf = small.tile([1, B], BF16, tag="lab_bf", name="lab_bf")
    labr_bf = small.tile([1, P], BF16, tag="labr_bf", name="labr_bf")
    lab_row_sb = small.tile([P, 1], F32, tag="lab_row_sb", name="lab_row_sb")
    tns = small.tile([P, NT], F32, tag="tns", name="tns")
    ty0 = small.tile([P, NT], F32, tag="ty0", name="ty0")
    trv = small.tile([P, NT], F32, tag="trv", name="trv")
    xt_scr = small.tile([P, C], F32, tag="xt_scr", name="xt_scr")
    gs = []

    chunk_idx = 1  # next chunk boundary to emit gram work for
    pending_cast = [None]

    def flush_cast():
        if pending_cast[0] is not None:
            a, b = pending_cast[0]
            nc.vector.tensor_copy(pooled_bf[:, a:b], pooled[:, a:b])
            pending_cast[0] = None

    def emit_gram_chunk(k):
        """Transpose pooled channels [chunks[k-1], chunks[k]) and accumulate
        their gram contribution into the psum g tiles."""
        c0, c1 = chunks[k - 1], chunks[k]
        w = c1 - c0
        # both matmul operands must share a base partition, so the chunk's
        # transpose lands at its natural offset within the 128-channel block
        cb = c0 // P
        r0 = c0 - cb * P
        pt = psum_tp.tile([P, P], F32, tag="pt", name="pt")
        nc.tensor.matmul(
            pt[r0 : r0 + w, :], pooled_bf[:, c0:c1], identity_bf, start=True, stop=True
        )
        last = k == len(chunks) - 1
        nc.vector.tensor_copy(pnT[k - 1][r0 : r0 + w, :], pt[r0 : r0 + w, :])
        if last and cfg.get("gap_filler", False):
            pw = psum_lab.tile([P, 512], F32, tag="pl", name="pl")
            nc.tensor.matmul(
                pw[:, :256], txtT[0][:, :P], txtT[0][:, :256], start=True, stop=True
            )
        first = k == 1
        for nbk in range(2):
            if first:
                g = psum_g.tile([P, 512], F32, tag=f"g{nbk}", name=f"g{nbk}")
                gs.append(g)
            nc.tensor.matmul(
                gs[nbk],
                pnT[k - 1][r0 : r0 + w, :],
                txtT[cb][r0 : r0 + w, nbk * 512 : (nbk + 1) * 512],
                start=first,
                stop=last,
            )

    # ---- per-tile extra work, keyed by tile index (emitted after that
    # tile's reduces so each engine queue stays ahead of the stream) ----
    def extra_work(t):
        if t == 0:
            # labels: on-chip broadcast. lab row already cast to bf16 below.
            nc.vector.tensor_copy(lab_bf, lab_all_sb)
            nc.vector.tensor_copy(labr_bf, lab_row_sb_row)
            pr = psum_tp.tile([P, P], F32, tag="pt", name="pt")
            nc.tensor.matmul(pr[:, :1], labr_bf, one1, start=True, stop=True)
            nc.vector.tensor_copy(lab_row_sb, pr[:, :1])
            for h in range(2):
                pl = psum_lab.tile([P, 512], F32, tag="pl", name="pl")
                nc.tensor.matmul(
                    pl, ones_row, lab_bf[:, h * 512 : (h + 1) * 512],
                    start=True, stop=True,
                )
                # tgt01 = (lab == lab_row); {0,1} so exact in bf16
                nc.vector.tensor_scalar(
                    out=tgt01[:, h * 512 : (h + 1) * 512],
                    in0=pl,
                    scalar1=lab_row_sb,
                    scalar2=1.0,
                    op0=ALU.is_equal,
                    op1=ALU.mult,
                )
        elif 1 <= t <= 4:
            # text norms, 2 tiles per img tile
            for tb in (2 * t - 2, 2 * t - 1):
                tsq = txtp.tile([P, C], F32, tag="tsq", name="tsq")
                nc.vector.tensor_mul(tsq, txt_sb[:, tb, :], txt_sb[:, tb, :])
                nc.vector.reduce_sum(out=tns[:, tb : tb + 1], in_=tsq, axis=AX.X)
        elif t == 5:
            # batched rsqrt for all 8 text tiles
            nc.scalar.activation(ty0, tns, ACT.Ln)
            nc.scalar.activation(ty0, ty0, ACT.Exp, scale=-0.5)
            _emit_newton(nc, small, ty0, tns, trv, "trsq", NT)
        elif 6 <= t <= 9:
            for tb in (2 * t - 12, 2 * t - 11):
                nc.vector.tensor_scalar_mul(
                    txt_bf[:, tb, :], txt_sb[:, tb, :], trv[:, tb : tb + 1]
                )
        elif 10 <= t <= 12:
            # text transposes: 16 matmuls over 3 tiles
            lo = (16 * (t - 10)) // 3
            hi = (16 * (t - 9)) // 3
            for i in range(lo, hi):
                tb, cb = divmod(i, 2)
                pt = psum_tp.tile([P, P], F32, tag="pt", name="pt")
                nc.tensor.matmul(
                    pt,
                    txt_bf[:, tb, cb * P : (cb + 1) * P],
                    identity_bf,
                    start=True,
                    stop=True,
                )
                nc.vector.tensor_copy(txtT[cb][:, tb * P : (tb + 1) * P], pt)
        elif t == 13:
            # transpose the target mask: tgtT[:, jt, b] = tgt01[b, jt*128+...]
            for jt in range(NT):
                pt = psum_tp.tile([P, P], F32, tag="pt", name="pt")
                nc.tensor.matmul(
                    pt,
                    tgt01[:, jt * P : (jt + 1) * P],
                    identity_bf,
                    start=True,
                    stop=True,
                )
                nc.vector.tensor_copy(tgtT[:, jt, :], pt)
        elif t == 14:
            # W[b,c] = sum_j tgt01[b,j] * txt_n[j,c]: needs NO img data, so it
            # runs mid-stream and takes the whole x*z term off the tail (the
            # tail then only needs sum_c pooled[b,c]*W[b,c]).
            pw = psum_warm.tile([P, C], F32, tag="W", name="W")
            for jt in range(NT):
                nc.tensor.matmul(
                    pw, tgtT[:, jt, :], txt_bf[:, jt, :],
                    start=(jt == 0), stop=(jt == NT - 1),
                )
            nc.vector.tensor_copy(w_sb, pw)

    # ---- the stream ----
    c = 0
    for t, sz in enumerate(sizes):
        it = big.tile([P, sz, HW], F32, tag="imgin", name="imgin")
        if t == ntiles - 1 and sz == 1 and cfg.get("split_last", True):
            # final channel arrives as two half-rows so the tail reduce can
            # start one half-transfer earlier
            nc.sync.dma_start(out=it[:, :, : HW // 2], in_=img[:, c : c + 1, : HW // 2])
            nc.sync.dma_start(out=it[:, :, HW // 2 :], in_=img[:, c : c + 1, HW // 2 :])
        else:
            nc.sync.dma_start(out=it, in_=img[:, c : c + sz, :], max_dma_last_dim=2304)
        if t == 0:
            # small inputs ride the stream behind tile 0
            # text + labels issue from the ACT ring: keeps the sync ring's
            # serial descriptor-issue pipeline exclusively for the img stream
            # (-4.6 us on the v1/CoreSim clock, neutral on TimelineSim)
            use_side = cfg.get("side_ring", True)
            side_t = nc.scalar if use_side in ("both", "txt", True) else nc.sync
            side_l = nc.scalar if use_side in ("both", "lab", True) else nc.sync
            if use_side == "lab3":
                side_t, side_l = nc.scalar, nc.vector
            side_t.dma_start(
                out=txt_sb, in_=txt.rearrange("(t p) c -> p t c", p=P)
            )
            lab_all_sb = small.tile([1, B], F32, tag="lab_all_sb", name="lab_all_sb")
            side_l.dma_start(out=lab_all_sb, in_=lab_all)
            lab_row_sb_row = small.tile([1, P], F32, tag="lab_row_r", name="lab_row_r")
            side_l.dma_start(out=lab_row_sb_row, in_=lab_row)
        c0 = c
        last_direct_bf = False
        for j in range(sz):
            chunk = it[:, j, :]
            # the very last channel reduce goes to DVE: its queue is empty at
            # the stream end, while ACT still owes squares + the rsqrt chain
            if c in pool_channels:
                nc.gpsimd.reduce_sum(out=pooled[:, c : c + 1], in_=chunk, axis=AX.X)
            elif (c % 16) in act_set and c != C - 1:
                ascr = ascrp.tile([P, HW], F32, tag="ascr", name="ascr")
                nc.scalar.activation(
                    ascr, chunk, ACT.Identity, accum_out=pooled[:, c : c + 1]
                )
            elif c == C - 1 and t == ntiles - 1 and sz == 1:
                ph = small.tile([P, 2], F32, tag="ph", name="ph")
                nc.vector.reduce_sum(out=ph[:, :1], in_=chunk[:, : HW // 2], axis=AX.X)
                nc.vector.reduce_sum(out=ph[:, 1:], in_=chunk[:, HW // 2 :], axis=AX.X)
                # fuse the half-combine with the bf16 cast: the transpose (the
                # tail's critical consumer) reads pooled_bf one op sooner; the
                # f32 copy for the norm/xt consumers runs off the critical path
                nc.vector.tensor_add(pooled_bf[:, c : c + 1], ph[:, :1], ph[:, 1:])
                nc.vector.tensor_add(pooled[:, c : c + 1], ph[:, :1], ph[:, 1:])
                last_direct_bf = True
            else:
                nc.vector.reduce_sum(out=pooled[:, c : c + 1], in_=chunk, axis=AX.X)
            c += 1
        # flush the PREVIOUS tile's bf16 cast now (deferred one tile so the
        # final tile's critical half-reduces aren't queued behind a cast);
        # also flushed before every chunk transpose to keep deps exact
        flush_cast()
        sq_scr = ascrp.tile([P, sz], F32, tag="sq_scr", name="sq_scr")
        nc.scalar.activation(
            sq_scr, pooled[:, c0:c], ACT.Square, accum_out=ns_parts[:, t : t + 1]
        )
        extra_work(t)
        if not last_direct_bf:
            pending_cast[0] = (c0, c)
        if t == cfg.get("warm_tile", ntiles - 4) and cfg.get("warm", False):
            # PE p-state warmers: two throwaway fp32 matmuls anchored on THIS
            # tile's freshly-DMA'd data, so they execute ~4us before the tail
            # matmuls and hold the PE ramp past the 3us full-speed threshold
            # (cold tail matmuls cost 427+788 ns; warm ones 213 each).
            for wi in range(2):
                pw = psum_lab.tile([P, 512], F32, tag="pl", name="pl")
                nc.tensor.matmul(
                    pw, it[:, 0, :P], it[:, 0, :512], start=True, stop=True
                )
        while chunk_idx < len(chunks) and c >= chunks[chunk_idx]:
            flush_cast()
            emit_gram_chunk(chunk_idx)
            chunk_idx += 1

    # ---- tail: pooled-row rsqrt + softplus/target accumulation ----
    ns = small.tile([P, 1], F32, tag="ns", name="ns")
    nc.vector.reduce_sum(out=ns, in_=ns_parts, axis=AX.X)
    # rsqrt via exp(-0.5*ln(ns)); no Newton step here — it sits on the tail's
    # critical path and the executor's Ln/Exp leave ~1e-5 rel error, far
    # inside the 2e-2 gate (the off-critical text rsqrt keeps its Newton).
    rv = small.tile([P, 1], F32, tag="rv", name="rv")
    nc.scalar.activation(rv, ns, ACT.Ln)
    nc.scalar.activation(rv, rv, ACT.Exp, scale=-0.5)
    rv_sc = small.tile([P, 1], F32, tag="rv_sc", name="rv_sc")
    nc.vector.tensor_scalar_mul(rv_sc, rv, INV_TEMP)

    # fin[:,0] = sum_j softplus(logits); fin[:,1] = sum_c pooled*W (raw x*z
    # term); fin[:,2] = rv. fin goes to HBM as-is; the host computes
    # sum(fin0) - (1/T)*sum(fin1*fin2) — exact, and it decouples the output
    # from the rsqrt chain while dropping an op from the tail.
    fin = small.tile([P, 3], F32, tag="fin", name="fin")
    nc.vector.tensor_copy(fin[:, 2:], rv)
    if cfg.get("escr_psum", False):
        e_scr = psum_warm.tile([P, B], F32, tag="e_scr", name="e_scr")
    else:
        e_scr = small.tile([P, B], F32, tag="e_scr", name="e_scr")
    for nbk in range(2):
        # softplus(x) = ln(exp(x) + 1); |x| <= 1/0.07 so exp can't overflow
        nc.scalar.activation(
            e_scr[:, nbk * 512 : (nbk + 1) * 512], gs[nbk], ACT.Exp, scale=rv_sc
        )
    # one wide Ln over both halves: saves an accumulator drain + op overhead
    if cfg.get("probe_noln", False):
        nc.vector.reduce_sum(out=fin[:, :1], in_=e_scr, axis=AX.X)
    else:
        nc.scalar.activation(e_scr, e_scr, ACT.Ln, bias=1.0, accum_out=fin[:, :1])
    if cfg.get("split_out", False):
        nc.sync.dma_start(out=out[:, 1:], in_=fin[:, 1:])
        nc.sync.dma_start(out=out[:, :1], in_=fin[:, :1])

    # emitted AFTER the Ln so the DVE ready-first scheduler prefers the
    # critical pnT copy over this off-path work at the stream tail
    nc.vector.tensor_mul(xt_scr, pooled, w_sb)
    nc.vector.reduce_sum(out=fin[:, 1:2], in_=xt_scr, axis=AX.X)
    if not cfg.get("probe_noout", False) and not cfg.get("split_out", False):
        out_q = nc.scalar if cfg.get("out_ring_act", False) else nc.sync
        out_q.dma_start(out=out, in_=fin)


def _build_nc(reps=1, **cfg):
    nc = bacc.Bacc("TRN2", target_bir_lowering=False, debug=False, num_devices=N_CORES)
    img = nc.dram_tensor("img", [BS, C, HW], F32, kind="ExternalInput").ap()
    txt = nc.dram_tensor("txt", [B, C], F32, kind="ExternalInput").ap()
    lab_row = nc.dram_tensor("lab_row", [1, BS], F32, kind="ExternalInput").ap()
    lab_all = nc.dram_tensor("lab_all", [1, B], F32, kind="ExternalInput").ap()
    outs = [
        nc.dram_tensor(
            "partial" if r == 0 else f"partial{r}", [P, 3], F32, kind="ExternalOutput"
        ).ap()
        for r in range(reps)
    ]

    with tile.TileContext(nc) as tc:
        with (
            tc.tile_pool(name="consts", bufs=1) as consts,
            tc.tile_pool(name="big", bufs=cfg.get("big_bufs", 6)) as big,
            tc.tile_pool(name="ascrp", bufs=2) as ascrp,
            tc.tile_pool(name="txtp", bufs=1) as txtp,
            tc.tile_pool(name="small", bufs=2) as small,
            tc.tile_pool(name="persist", bufs=1) as persist,
            tc.tile_pool(name="psum_tp", bufs=2, space="PSUM") as psum_tp,
            tc.tile_pool(name="psum_g", bufs=1, space="PSUM") as psum_g,
            tc.tile_pool(name="psum_lab", bufs=1, space="PSUM") as psum_lab,
            tc.tile_pool(name="psum_warm", bufs=1, space="PSUM") as psum_warm,
        ):
            identity_bf = consts.tile([P, P], BF16, tag="identity")
            make_identity(nc, identity_bf)
            ones_row = consts.tile([1, P], BF16, tag="ones_row")
            nc.vector.memset(ones_row, 1.0)
            one1 = consts.tile([1, 1], BF16, tag="one1")
            nc.vector.memset(one1, 1.0)
            pools = (consts, big, ascrp, txtp, small, persist, psum_tp, psum_g, psum_lab, psum_warm)
            for r in range(reps):
                _emit_body(
                    nc, pools, identity_bf, ones_row, one1,
                    img, txt, lab_row, lab_all, outs[r], cfg,
                )

    nc.finalize()
    return nc


def _get_nc():
    if not _NC_CACHE:
        _NC_CACHE.append(_build_nc())
    return _NC_CACHE[0]


def make_in_maps(img_features, text_embeds, labels_f):
    img3 = img_features.reshape(B, C, HW)
    in_maps = []
    for i in range(N_CORES):
        sl = slice(i * BS, (i + 1) * BS)
        in_maps.append(
            {
                "img": img3[sl],
                "txt": text_embeds,
                "lab_row": labels_f[sl].reshape(1, BS),
                "lab_all": labels_f.reshape(1, B),
            }
        )
    return in_maps


def kernel(img_features, text_embeds, labels):
    img_features = np.ascontiguousarray(np.asarray(img_features, dtype=np.float32))
    text_embeds = np.ascontiguousarray(np.asarray(text_embeds, dtype=np.float32))
    labels_f = np.asarray(labels).astype(np.float32)  # values < 16: exact in f32

    nc = _get_nc()
    in_maps = make_in_maps(img_features, text_embeds, labels_f)
    r = run_bass_kernel_spmd(nc, in_maps, core_ids=list(range(N_CORES)))
    total = 0.0
    for i in range(N_CORES):
        p = r.results[i]["partial"].astype(np.float64)
        total += p[:, 0].sum() - INV_TEMP * float(p[:, 1] @ p[:, 2])
    return np.float32(total / (B * B))
